# revision 1
# baseline (speedup 1.0000x reference)
"""CharacterAwareEncoder kernel for Trainium2 (8 NeuronCores, data-parallel).

reference:
    word_embeds  = word_emb_table[word_ids]                  # [B, S, 412] gather
    char_features = sin(freqs * word_ids), 0 where id == 0   # [B, S, 100]
    out = concat([word_embeds, char_features], -1)           # [B, S, 512]

Sharding: word_ids [16, 2048] flattened to 32768 tokens, 4096 per core;
embedding table replicated. Per core: 32 tiles of 128 tokens; each tile's
rows are gathered straight into the first 412 columns of a [128, 512]
output slice via indirect DMA, the sinusoidal features are computed with
a Cody-Waite range reduction + ACT-engine Sin into the last 100 columns,
and the fused [128, 512] rows are stored contiguously.

sin accuracy: x = freq*tok <= 3168 rad.  k = int(x / 2pi) (either trunc or
round-to-nearest hardware cast works), r = ((x - k*c1) - k*c2) - k*c3 with a
3-term Cody-Waite split of 2pi, then a +-2pi range wrap (fixes any off-by-one
k) and a clamp to +-PI_SAFE so the ACT Sin table (valid on [-pi, pi]) never
sees an out-of-domain value.  Max abs error vs float64 sin ~4e-7.
"""

import numpy as np

import concourse.bacc as bacc
import concourse.bass as bass
import concourse.mybir as mybir
import concourse.tile as tile
from concourse.bass_utils import run_bass_kernel_spmd

B, S = 16, 2048
V, D, H = 32000, 412, 100
OUT_D = 512
N_CORES = 8
P = 128
T_CORE = B * S // N_CORES          # 4096 tokens per core
N_TILES = T_CORE // P              # 32 tiles of 128 tokens
CHUNK_TILES = 2                    # tiles per double-buffered SBUF chunk
N_CHUNKS = N_TILES // CHUNK_TILES  # 16
SIN_TILES = 8                      # tiles per wide sin-pipeline block
N_SIN_BLOCKS = N_TILES // SIN_TILES  # 4
W = SIN_TILES * H                  # sin-pipeline width per block (800)

_f32 = mybir.dt.float32
_i32 = mybir.dt.int32

# Cody-Waite split of 2*pi: c1/c2 keep 12 mantissa bits so k*c1, k*c2 are
# exact for k <= 505; c3 absorbs the rest (residual ~7e-15).
_TWO_PI = 2.0 * np.pi
def _split_high(v):
    f = np.float32(v)
    return (f.view(np.uint32) & np.uint32(0xFFFFF000)).view(np.float32)
C1 = float(_split_high(_TWO_PI))
C2 = float(_split_high(_TWO_PI - C1))
C3 = float(np.float32(_TWO_PI - C1 - C2))
INV2PI = float(np.float32(1.0 / _TWO_PI))
PI_F32 = float(np.float32(np.pi))
TWO_PI_F32 = float(np.float32(_TWO_PI))
PI_SAFE = float(np.nextafter(np.float32(np.pi), np.float32(0)))  # < float64 pi

_NC = {}

# "indirect": one generic SWDGE indirect DMA per 128-token tile, unpadded
#   table rows (1648 B each).
# "dma_gather": one custom InstDMAGatherAnt per chunk, table padded to 512
#   floats/row (2048 B) on host so elem_size_bytes % 256 == 0; int16 indices
#   in the wrapped [i%16, i//16] layout replicated over 8x16 partitions.
GATHER_MODE = "indirect"
SWDGE_QUEUES = 2  # split indirect gathers across SWDGE queues (parallel Q7 desc-gen)
N_PASSES = 1  # >1 only for repeat-amplification timing probes
_i16 = mybir.dt.int16


def _build_nc(mode=None):
    mode = mode or GATHER_MODE
    # Bacc (not plain Bass): its compile() pass splits multi-semaphore waits
    # into InstEventSemaphore chains — TRN2 compute instructions encode at
    # most one sync wait, and walrus refuses to legalize this itself.
    nc = bacc.Bacc("TRN2", target_bir_lowering=False,
                   num_swdge_queues=SWDGE_QUEUES)
    # consts: [0:W] freqs tiled CHUNK_TILES times, [W:W+N_TILES] token ids as f32
    consts_t = nc.dram_tensor("consts", [P, W + N_TILES], _f32, kind="ExternalInput")
    if mode == "indirect":
        ids_t = nc.dram_tensor("ids", [P, N_TILES], _i32, kind="ExternalInput")
        table_t = nc.dram_tensor("table", [V, D], _f32, kind="ExternalInput")
    elif mode == "hybrid":
        ids_t = nc.dram_tensor("ids", [P, N_TILES], _i32, kind="ExternalInput")
        idx16_t = nc.dram_tensor("idx16", [P, T_CORE // 16], _i16, kind="ExternalInput")
        table_t = nc.dram_tensor("table", [V, OUT_D], _f32, kind="ExternalInput")
    else:
        ids_t = nc.dram_tensor("idx16", [P, T_CORE // 16], _i16, kind="ExternalInput")
        table_t = nc.dram_tensor("table", [V, OUT_D], _f32, kind="ExternalInput")
    out_t = nc.dram_tensor("out", [T_CORE, OUT_D], _f32, kind="ExternalOutput")

    with tile.TileContext(nc) as tc:
        with (
            tc.tile_pool(name="const", bufs=1) as cpool,
            tc.tile_pool(name="chunks", bufs=6) as chpool,
            tc.tile_pool(name="bigch", bufs=2) as bigpool,
            tc.tile_pool(name="work", bufs=2) as wpool,
        ):
            if mode == "indirect":
                ids_sb = cpool.tile([P, N_TILES], _i32)
            elif mode == "hybrid":
                ids_sb = cpool.tile([P, N_TILES], _i32)
                idx16_sb = cpool.tile([P, T_CORE // 16], _i16)
                nc.sync.dma_start(out=idx16_sb[:], in_=idx16_t[:])
            else:
                ids_sb = cpool.tile([P, T_CORE // 16], _i16)
            nc.sync.dma_start(out=ids_sb[:], in_=ids_t[:])
            consts_sb = cpool.tile([P, W + N_TILES], _f32)
            nc.sync.dma_start(out=consts_sb[:], in_=consts_t[:])
            freqs_sb = consts_sb[:, 0:W]
            tokf = consts_sb[:, W : W + N_TILES]

            chunk_toks = CHUNK_TILES * P

            def emit_sin_block(start_tile, n_tiles):
                """6-op DVE range-reduction pipeline for n_tiles tiles.

                DVE per-op fixed costs favor wide blocks, but a wide first
                block delays the pipeline head — callers mix widths."""
                w = n_tiles * H
                tok_b = tokf[:, start_tile : start_tile + n_tiles]
                x = wpool.tile([P, W], _f32, tag="x")
                nc.vector.tensor_tensor(
                    out=x[:, 0:w].rearrange("p (j h) -> p j h", j=n_tiles),
                    in0=tok_b.to_broadcast([P, n_tiles, H]),
                    in1=freqs_sb[:, 0:w].rearrange("p (j h) -> p j h", j=n_tiles),
                    op=mybir.AluOpType.mult,
                )
                kint = wpool.tile([P, W], _i32, tag="kint")
                nc.vector.tensor_scalar(
                    out=kint[:, 0:w], in0=x[:, 0:w], scalar1=INV2PI, scalar2=None,
                    op0=mybir.AluOpType.mult,
                )
                kf = wpool.tile([P, W], _f32, tag="kf")
                nc.vector.tensor_copy(out=kf[:, 0:w], in_=kint[:, 0:w])
                r = wpool.tile([P, W], _f32, tag="r")
                nc.vector.cody_waite_cascade(
                    out=r[:, 0:w], x=x[:, 0:w], k=kf[:, 0:w], c1=C1, c2=C2, c3=C3
                )
                r2 = wpool.tile([P, W], _f32, tag="r2")
                nc.vector.add_range_wrap(
                    out=r2[:, 0:w], in_=r[:, 0:w], shift=0.0, bound=PI_F32,
                    period=TWO_PI_F32,
                )
                r3 = wpool.tile([P, W], _f32, tag="r3")
                nc.vector.tensor_scalar(
                    out=r3[:, 0:w], in0=r2[:, 0:w], scalar1=PI_SAFE, scalar2=-PI_SAFE,
                    op0=mybir.AluOpType.min, op1=mybir.AluOpType.max,
                )
                return r3

            def emit_pass():
              # narrow blocks first to prime the pipeline, wide after
              sin_plan = [CHUNK_TILES] * (SIN_TILES // CHUNK_TILES)
              while sum(sin_plan) < N_TILES:
                sin_plan.append(SIN_TILES)
              tile_block = {}  # start tile of chunk -> (r3 tile, block start)
              blocks_emitted = 0
              next_block_tile = 0

              for g in range(N_CHUNKS):
                tile0 = g * CHUNK_TILES
                if tile0 == next_block_tile:
                    n_t = sin_plan[blocks_emitted]
                    r3_b = emit_sin_block(tile0, n_t)
                    for tt in range(tile0, tile0 + n_t, CHUNK_TILES):
                        tile_block[tt] = (r3_b, tile0)
                    blocks_emitted += 1
                    next_block_tile += n_t

                ch = chpool.tile([P, CHUNK_TILES, OUT_D], _f32, tag="ch")
                if mode == "indirect":
                    # One gather per 128-token tile. NOTE: a single batched
                    # indirect DMA with a [128, k] offset AP matches CoreSim
                    # but is WRONG on hardware (the DGE reads consecutive
                    # table rows past the first offset of each partition) —
                    # keep offsets strictly [128, 1] per instruction.
                    for j in range(CHUNK_TILES):
                        t = g * CHUNK_TILES + j
                        gi = nc.gpsimd.indirect_dma_start(
                            out=ch[:, j, 0:D],
                            out_offset=None,
                            in_=table_t[:],
                            in_offset=bass.IndirectOffsetOnAxis(
                                ap=ids_sb[:, t : t + 1], axis=0
                            ),
                        )
                        if SWDGE_QUEUES > 1 and t % SWDGE_QUEUES:
                            gi.queue = f"qPoolDynamic{t % SWDGE_QUEUES}"
                else:
                    # One custom-ucode gather for the whole chunk:
                    # dst[i%128, i//128, :] = table[idx[i], :] for the
                    # chunk's 512 tokens — exactly the ch layout. The padded
                    # columns 412:512 are overwritten by the sin below.
                    nc.gpsimd.dma_gather(
                        ch[:],
                        table_t[:],
                        ids_sb[:, g * (chunk_toks // 16) : (g + 1) * (chunk_toks // 16)],
                        chunk_toks,
                        chunk_toks,
                        OUT_D,
                    )

                r3_b, bstart = tile_block[g * CHUNK_TILES]
                jj = g * CHUNK_TILES - bstart
                nc.scalar.activation(
                    out=ch[:, :, D:OUT_D],
                    in_=r3_b[:, jj * H : (jj + CHUNK_TILES) * H]
                    .rearrange("p (j h) -> p j h", j=CHUNK_TILES),
                    func=mybir.ActivationFunctionType.Sin,
                )

                # store: token g*CT*128 + j*128 + p lives at ch[p, j, :].
                # Alternate the two HWDGE rings (SP via nc.sync, ACT via
                # nc.scalar) so descriptor generation isn't serialized on
                # one engine.
                store_eng = nc.sync if g % 2 == 0 else nc.scalar
                store_eng.dma_start(
                    out=out_t[g * CHUNK_TILES * P : (g + 1) * CHUNK_TILES * P, :]
                    .rearrange("(j p) c -> p j c", p=P),
                    in_=ch[:],
                )

            def emit_pass_hybrid():
              # Tiles 0..15: fine-grained indirect chunks (full padded rows).
              # Tiles 16..31: two 1024-row dma_gather super-chunks — cuts the
              # Pool engine's serial gather dispatches from 32 to 18.
              sin_plan = [CHUNK_TILES] * (SIN_TILES // CHUNK_TILES)
              while sum(sin_plan) < N_TILES // 2:
                sin_plan.append(SIN_TILES)
              tile_block = {}
              blocks_emitted = 0
              next_block_tile = 0
              for g in range((N_TILES // 2) // CHUNK_TILES):
                tile0 = g * CHUNK_TILES
                if tile0 == next_block_tile:
                    n_t = sin_plan[blocks_emitted]
                    r3_b = emit_sin_block(tile0, n_t)
                    for tt in range(tile0, tile0 + n_t, CHUNK_TILES):
                        tile_block[tt] = (r3_b, tile0)
                    blocks_emitted += 1
                    next_block_tile += n_t
                ch = chpool.tile([P, CHUNK_TILES, OUT_D], _f32, tag="ch")
                for j in range(CHUNK_TILES):
                    t = tile0 + j
                    gi = nc.gpsimd.indirect_dma_start(
                        out=ch[:, j, :],
                        out_offset=None,
                        in_=table_t[:],
                        in_offset=bass.IndirectOffsetOnAxis(
                            ap=ids_sb[:, t : t + 1], axis=0
                        ),
                    )
                    if SWDGE_QUEUES > 1 and t % SWDGE_QUEUES:
                        gi.queue = f"qPoolDynamic{t % SWDGE_QUEUES}"
                r3_b, bstart = tile_block[tile0]
                jj = tile0 - bstart
                nc.scalar.activation(
                    out=ch[:, :, D:OUT_D],
                    in_=r3_b[:, jj * H : (jj + CHUNK_TILES) * H]
                    .rearrange("p (j h) -> p j h", j=CHUNK_TILES),
                    func=mybir.ActivationFunctionType.Sin,
                )
                store_eng = nc.sync if g % 2 == 0 else nc.scalar
                store_eng.dma_start(
                    out=out_t[tile0 * P : (tile0 + CHUNK_TILES) * P, :]
                    .rearrange("(j p) c -> p j c", p=P),
                    in_=ch[:],
                )
              sc_toks = SIN_TILES * P
              for s in range((N_TILES // 2) // SIN_TILES):
                tile0 = N_TILES // 2 + s * SIN_TILES
                big = bigpool.tile([P, SIN_TILES, OUT_D], _f32, tag="big")
                nc.gpsimd.dma_gather(
                    big[:],
                    table_t[:],
                    idx16_sb[:, tile0 * P // 16 : (tile0 * P + sc_toks) // 16],
                    sc_toks,
                    sc_toks,
                    OUT_D,
                )
                r3_b = emit_sin_block(tile0, SIN_TILES)
                nc.scalar.activation(
                    out=big[:, :, D:OUT_D],
                    in_=r3_b[:, 0 : SIN_TILES * H]
                    .rearrange("p (j h) -> p j h", j=SIN_TILES),
                    func=mybir.ActivationFunctionType.Sin,
                )
                # split the 2MB store into 512KB sub-stores alternating both
                # HWDGE rings — one big store serializes ~6.3us on one ring
                # at the kernel tail
                for q in range(0, SIN_TILES, CHUNK_TILES):
                    r0 = (tile0 + q) * P
                    eng = nc.sync if (s + q // CHUNK_TILES) % 2 == 0 else nc.scalar
                    eng.dma_start(
                        out=out_t[r0 : r0 + CHUNK_TILES * P, :]
                        .rearrange("(j p) c -> p j c", p=P),
                        in_=big[:, q : q + CHUNK_TILES, :],
                    )

            for _ in range(N_PASSES):
                if mode == "hybrid":
                    emit_pass_hybrid()
                else:
                    emit_pass()
    nc.compile()
    return nc


def _get_nc(mode=None):
    mode = mode or GATHER_MODE
    if mode not in _NC:
        _NC[mode] = _build_nc(mode)
    return _NC[mode]


def make_in_maps(word_ids, word_emb_table, mode=None):
    mode = mode or GATHER_MODE
    ids = np.ascontiguousarray(np.asarray(word_ids)).astype(np.int32).reshape(-1)
    table = np.ascontiguousarray(np.asarray(word_emb_table, dtype=np.float32))
    if mode != "indirect":
        padded = np.zeros((V, OUT_D), np.float32)
        padded[:, 0:D] = table
        table = padded
    freqs_row = np.tile(np.arange(H, dtype=np.float32) / np.float32(1000.0),
                        W // H)  # [W]

    in_maps = []
    for c in range(N_CORES):
        shard = ids[c * T_CORE : (c + 1) * T_CORE]
        ids_in = np.ascontiguousarray(shard.reshape(N_TILES, P).T)  # [P, N_TILES]
        consts = np.empty((P, W + N_TILES), np.float32)
        consts[:, 0:W] = freqs_row
        consts[:, W:] = ids_in.astype(np.float32)  # exact, ids < 2^24
        m = {"consts": consts, "table": table}
        if mode in ("indirect", "hybrid"):
            m["ids"] = ids_in
        if mode != "indirect":
            # wrapped int16 layout: shard token i at [i % 16, i // 16],
            # replicated over the 8 groups of 16 partitions (one per Q7 core)
            base = shard.astype(np.int16).reshape(T_CORE // 16, 16).T  # [16, n/16]
            m["idx16"] = np.ascontiguousarray(np.tile(base, (8, 1)))
        in_maps.append(m)
    return in_maps


def kernel(word_ids, word_emb_table):
    nc = _get_nc()
    in_maps = make_in_maps(word_ids, word_emb_table)
    res = run_bass_kernel_spmd(nc, in_maps, core_ids=list(range(N_CORES)))
    out = np.concatenate([r["out"] for r in res.results], axis=0)
    return out.reshape(B, S, OUT_D)



# revision 24
# speedup vs baseline: 1.6244x; 1.6244x over previous
"""CharacterAwareEncoder kernel for Trainium2 (8 NeuronCores, data-parallel).

reference:
    word_embeds  = word_emb_table[word_ids]                  # [B, S, 412] gather
    char_features = sin(freqs * word_ids), 0 where id == 0   # [B, S, 100]
    out = concat([word_embeds, char_features], -1)           # [B, S, 512]

Sharding: word_ids [16, 2048] flattened to 32768 tokens, 4096 per core;
embedding table replicated (padded to 448 f32/row on host so each gathered
row is one 1792-B descriptor, the smallest 256-B-aligned row covering 412).

Per core, four independent lanes (CoreSim charges each DMA's transfer as an
exclusive hold on its issuing engine; transfers on different engines overlap
freely):
  Pool  - the SWDGE gather stream: 10 InstDMAGatherAnt chunks cover all 4096
          rows at ~2.92 ns/token into one [128, 32, 448] SBUF arena, plus the
          idx load and the tail embed stores.
  DVE   - sin range reduction: x = tok*freq (freqs broadcast from a single
          [128, 100] column block); y = x*INV2PI + 1.5*2^23 (magic
          round-to-nearest-even, replaces int casts and the range wrap);
          kf = y - MAGIC; r = Cody-Waite cascade; clamp to +-PI_SAFE.
          Blocks descend [10, 10, 10, 2] tiles so the last sin + sin store
          chain off the DVE tail is short.
  ACT   - the Sin activations plus mid-kernel embed stores; its queue after
          the last Sin holds only the tiny last sin store.
  SP    - consts load and the bulk of the embed stores.

The output is produced as two DRAM tensors - out_emb [4096, 412] (gather
layout, token = j*128 + p) and out_sin [4096, 100] (p-major layout,
token = p*32 + j, so each partition stores contiguous multi-KB runs) -
and the host concatenates columns during unsharding.

sin accuracy: x = freq*tok <= 3168 rad. k = RNE(x/2pi) exactly via magic
add/sub, r = ((x - k*c1) - k*c2) - k*c3 (Cody-Waite), clamp to +-PI_SAFE
so the ACT Sin table never sees |x| > pi. Worst case ~2e-4 abs err on the
~6e-5 fraction of elements within float rounding of an odd multiple of pi
(k off by one, |r| marginally > pi, clamped); everywhere else ~4e-7.
"""

import numpy as np

import concourse.bacc as bacc
import concourse.bass as bass
import concourse.mybir as mybir
import concourse.tile as tile
from concourse.bass_utils import run_bass_kernel_spmd

B, S = 16, 2048
V, D, H = 32000, 412, 100
OUT_D = 512
GW = 448                           # gathered row width (256-B aligned >= 412)
N_CORES = 8
P = 128
T_CORE = B * S // N_CORES          # 4096 tokens per core
N_TILES = T_CORE // P              # 32 tiles of 128 tokens
JPP = T_CORE // P                  # sin tokens per partition (p-major), 32

# gather chunks in tiles (sum = 32): small head chunk starts the store lanes
# early, small tail chunks keep the final chain short.
CHUNK_TILES = [2, 4, 4, 4, 4, 4, 4, 3, 2, 1]
# sin-pipeline blocks in tiles: descending so the last blocks' range
# reduction, Sin, and sin store are all tiny.
BLK_TILES = [10, 10, 6, 4, 2]
WMAX = 16 * H

# schedule: interleaved emission program. Entries:
#   ("g", chunk_idx)            gather on Pool
#   ("blk", b)                  DVE range-reduction block b
#   ("sin", b)                  ACT Sin for block b
#   ("e", eng, t0, t1)          embed store tiles [t0, t1)
#   ("s", eng, b)               sin store for block b
# Best-found schedule (CoreSim 20236 ns/core): kf of block 0 rides the ACT
# Copy activation (fills ACT's pre-sin idle, shortens the DVE chain); sin
# stores for the two big blocks land on SP/Pool; mid/late embed stores are
# <=2-tile pieces so no lane grabs a multi-us job right before a
# tail-critical Sin or sin store becomes ready.
SCHEDULE = (
    [("blk", 0, True), ("sin", 0), ("blk", 1), ("sin", 1), ("blk", 2),
     ("sin", 2), ("blk", 3), ("sin", 3), ("blk", 4), ("sin", 4),
     ("s", "sp", 0), ("s", "pool", 1), ("s", "sp", 2), ("s", "pool", 3),
     ("s", "sp", 4),
     ("e", "sp", 0, 2), ("e", "sp", 2, 6), ("e", "sp", 6, 10),
     ("e", "act", 10, 12), ("e", "act", 12, 14), ("e", "sp", 14, 16),
     ("e", "sp", 16, 18), ("e", "act", 18, 20), ("e", "act", 20, 22),
     ("e", "sp", 22, 24), ("e", "sp", 24, 26), ("e", "act", 26, 28),
     ("e", "pool", 28, 29), ("e", "pool", 29, 31), ("e", "pool", 31, 32)]
)

_f32 = mybir.dt.float32
_i16 = mybir.dt.int16

_TWO_PI = 2.0 * np.pi
def _split_high(v):
    f = np.float32(v)
    return (f.view(np.uint32) & np.uint32(0xFFFFF000)).view(np.float32)
C1 = float(_split_high(_TWO_PI))
C2 = float(_split_high(_TWO_PI - C1))
C3 = float(np.float32(_TWO_PI - C1 - C2))
INV2PI = float(np.float32(1.0 / _TWO_PI))
MAGIC = float(np.float32(1.5 * 2.0**23))  # RNE quantizer for |y| < 2^22
PI_SAFE = float(np.nextafter(np.float32(np.pi), np.float32(0)))

GATHER_MODE = "v5"
_NC = {}


def _build_nc(mode=None, chunk_tiles=None, blk_tiles=None, schedule=None,
              consts_on_pool=False):
    chunk_tiles = chunk_tiles or CHUNK_TILES
    blk_tiles = blk_tiles or BLK_TILES
    schedule = schedule or SCHEDULE
    blk_start = np.cumsum([0] + list(blk_tiles))
    nc = bacc.Bacc("TRN2", target_bir_lowering=False, num_swdge_queues=1)
    # consts: [0:H] freqs, [H:H+JPP] p-major token ids as f32
    consts_t = nc.dram_tensor("consts", [P, H + JPP], _f32,
                              kind="ExternalInput")
    idx_t = nc.dram_tensor("idx16", [P, T_CORE // 16], _i16,
                           kind="ExternalInput")
    table_t = nc.dram_tensor("table", [V, GW], _f32, kind="ExternalInput")
    oemb_t = nc.dram_tensor("out_emb", [T_CORE, D], _f32,
                            kind="ExternalOutput")
    osin_t = nc.dram_tensor("out_sin", [T_CORE, H], _f32,
                            kind="ExternalOutput")

    chunk_start = np.cumsum([0] + list(chunk_tiles))

    with tile.TileContext(nc) as tc:
        with (
            tc.tile_pool(name="const", bufs=1) as cpool,
            tc.tile_pool(name="arena", bufs=1) as apool,
            tc.tile_pool(name="work", bufs=2) as wpool,
        ):
            idx_sb = cpool.tile([P, T_CORE // 16], _i16)
            consts_sb = cpool.tile([P, H + JPP], _f32)
            if consts_on_pool:
                # consts first on Pool: the sin pipeline's gate loads before
                # idx; same-engine SWDGE ordering lets the gathers follow
                # the idx write without a semaphore round-trip.
                nc.gpsimd.dma_start(out=consts_sb[:], in_=consts_t[:])
                nc.gpsimd.dma_start(out=idx_sb[:], in_=idx_t[:])
            else:
                # idx via Pool SWDGE: tiny engine hold, and the gather
                # stream engine owns its own critical input.
                nc.gpsimd.dma_start(out=idx_sb[:], in_=idx_t[:])
                nc.sync.dma_start(out=consts_sb[:], in_=consts_t[:])
            freqs_sb = consts_sb[:, 0:H]
            tokf = consts_sb[:, H : H + JPP]

            ch = apool.tile([P, N_TILES, GW], _f32)    # gathered rows
            r3 = apool.tile([P, JPP, H], _f32)         # clamped angles
            sinout = apool.tile([P, JPP, H], _f32)     # sin values (p-major)
            scratch = apool.tile([P, 1], _f32)

            # ACT warmup: force the Sin act-table load during the idle head.
            nc.vector.memset(scratch[:], 0.0)
            nc.scalar.activation(out=scratch[:], in_=scratch[:],
                                 func=mybir.ActivationFunctionType.Sin)

            def emit_gather(c):
                t0, t1 = chunk_start[c], chunk_start[c + 1]
                toks = (t1 - t0) * P
                nc.gpsimd.dma_gather(
                    ch[:, t0:t1, :],
                    table_t[:],
                    idx_sb[:, t0 * (P // 16) : t1 * (P // 16)],
                    toks, toks, GW,
                )

            def emit_block(b, kf_on_act=False):
                """DVE: x, y, kf, Cody-Waite, clamp for block b.
                kf_on_act routes the kf subtraction through the ACT Copy
                activation (fills ACT's pre-sin idle, shortens DVE)."""
                j0, j1 = blk_start[b], blk_start[b + 1]
                nt = j1 - j0
                w = nt * H
                tb = tokf[:, j0:j1]
                x = wpool.tile([P, WMAX], _f32, tag="x")
                nc.vector.tensor_tensor(
                    out=x[:, 0:w].rearrange("p (j h) -> p j h", j=nt),
                    in0=tb.to_broadcast([P, nt, H]),
                    in1=freqs_sb.rearrange("p (j h) -> p j h", j=1)
                    .to_broadcast([P, nt, H]),
                    op=mybir.AluOpType.mult,
                )
                y = wpool.tile([P, WMAX], _f32, tag="y")
                nc.vector.tensor_scalar(
                    out=y[:, 0:w], in0=x[:, 0:w], scalar1=INV2PI,
                    scalar2=MAGIC,
                    op0=mybir.AluOpType.mult, op1=mybir.AluOpType.add,
                )
                kf = wpool.tile([P, WMAX], _f32, tag="kf")
                if kf_on_act:
                    nc.scalar.activation(
                        out=kf[:, 0:w], in_=y[:, 0:w],
                        func=mybir.ActivationFunctionType.Copy,
                        scale=1.0, bias=-MAGIC,
                    )
                else:
                    nc.vector.tensor_scalar(
                        out=kf[:, 0:w], in0=y[:, 0:w], scalar1=-MAGIC,
                        scalar2=None, op0=mybir.AluOpType.add,
                    )
                r = wpool.tile([P, WMAX], _f32, tag="r")
                nc.vector.cody_waite_cascade(
                    out=r[:, 0:w], x=x[:, 0:w], k=kf[:, 0:w],
                    c1=C1, c2=C2, c3=C3,
                )
                nc.vector.tensor_scalar(
                    out=r3[:, j0:j1, :],
                    in0=r[:, 0:w].rearrange("p (j h) -> p j h", j=nt),
                    scalar1=PI_SAFE, scalar2=-PI_SAFE,
                    op0=mybir.AluOpType.min, op1=mybir.AluOpType.max,
                )

            def emit_sin(b):
                j0, j1 = blk_start[b], blk_start[b + 1]
                nc.scalar.activation(
                    out=sinout[:, j0:j1, :],
                    in_=r3[:, j0:j1, :],
                    func=mybir.ActivationFunctionType.Sin,
                )

            ENG = {"sp": nc.sync, "act": nc.scalar, "pool": nc.gpsimd}

            def emit_sstore(eng, b):
                # p-major: one contiguous multi-KB run per partition
                j0, j1 = blk_start[b], blk_start[b + 1]
                ENG[eng].dma_start(
                    out=osin_t[:].rearrange("(p j) c -> p j c", p=P)
                    [:, j0:j1, :],
                    in_=sinout[:, j0:j1, :],
                )

            def emit_estore(eng, t0, t1):
                ENG[eng].dma_start(
                    out=oemb_t[t0 * P : t1 * P, :]
                    .rearrange("(j p) c -> p j c", p=P),
                    in_=ch[:, t0:t1, 0:D],
                )

            # ---- emission (priority = emission order for the tile
            # scheduler; lane = issuing engine) ----
            for c in range(len(chunk_tiles)):
                emit_gather(c)                   # Pool stream
            for step in schedule:
                if step[0] == "blk":
                    emit_block(step[1], *step[2:])
                elif step[0] == "sin":
                    emit_sin(step[1])
                elif step[0] == "e":
                    emit_estore(step[1], step[2], step[3])
                elif step[0] == "s":
                    emit_sstore(step[1], step[2])
                else:
                    raise ValueError(step)
    nc.compile()
    return nc


def _get_nc(mode=None):
    if "v5" not in _NC:
        _NC["v5"] = _build_nc()
    return _NC["v5"]


def make_in_maps(word_ids, word_emb_table, mode=None):
    ids = np.ascontiguousarray(np.asarray(word_ids)).astype(np.int32).reshape(-1)
    table = np.asarray(word_emb_table, dtype=np.float32)
    padded = np.zeros((V, GW), np.float32)
    padded[:, 0:D] = table
    freqs_row = np.arange(H, dtype=np.float32) / np.float32(1000.0)

    in_maps = []
    for c in range(N_CORES):
        shard = ids[c * T_CORE : (c + 1) * T_CORE]
        consts = np.empty((P, H + JPP), np.float32)
        consts[:, 0:H] = freqs_row
        # p-major token layout for the sin pipeline: tok (p, j) = shard[p*JPP+j]
        consts[:, H:] = shard.reshape(P, JPP).astype(np.float32)
        # wrapped int16 layout for dma_gather: token i at [i % 16, i // 16],
        # replicated over the 8 groups of 16 partitions (one per Q7 core)
        base = shard.astype(np.int16).reshape(T_CORE // 16, 16).T  # [16, n/16]
        in_maps.append({
            "consts": consts,
            "table": padded,
            "idx16": np.ascontiguousarray(np.tile(base, (8, 1))),
        })
    return in_maps


def kernel(word_ids, word_emb_table):
    nc = _get_nc()
    in_maps = make_in_maps(word_ids, word_emb_table)
    res = run_bass_kernel_spmd(nc, in_maps, core_ids=list(range(N_CORES)))
    outs = []
    for r in res.results:
        emb = r["out_emb"]                       # [T_CORE, 412], token-major
        sin = r["out_sin"]                       # [T_CORE, 100], token-major
        outs.append(np.concatenate([emb, sin], axis=1))
    return np.concatenate(outs, axis=0).reshape(B, S, OUT_D)


# revision 29
# speedup vs baseline: 1.6771x; 1.0324x over previous
"""CharacterAwareEncoder kernel for Trainium2 (8 NeuronCores, data-parallel).

reference:
    word_embeds  = word_emb_table[word_ids]                  # [B, S, 412] gather
    char_features = sin(freqs * word_ids), 0 where id == 0   # [B, S, 100]
    out = concat([word_embeds, char_features], -1)           # [B, S, 512]

Sharding: word_ids [16, 2048] flattened to 32768 tokens, 4096 per core;
embedding table replicated (padded to 448 f32/row on host so each gathered
row is one 1792-B descriptor, the smallest 256-B-aligned row covering 412).

Per core, four independent lanes (CoreSim charges each DMA's transfer as an
exclusive hold on its issuing engine; transfers on different engines overlap
freely):
  Pool  - the SWDGE gather stream: 10 InstDMAGatherAnt chunks cover all 4096
          rows at ~2.92 ns/token into one [128, 32, 448] SBUF arena, plus the
          idx load and the tail embed stores.
  DVE   - sin range reduction: x = tok*freq (freqs broadcast from a single
          [128, 100] column block); y = x*INV2PI + 1.5*2^23 (magic
          round-to-nearest-even, replaces int casts and the range wrap);
          kf = y - MAGIC; r = Cody-Waite cascade; clamp to +-PI_SAFE.
          Blocks descend [10, 10, 10, 2] tiles so the last sin + sin store
          chain off the DVE tail is short.
  ACT   - the Sin activations plus mid-kernel embed stores; its queue after
          the last Sin holds only the tiny last sin store.
  SP    - consts load and the bulk of the embed stores.

The output is produced as two DRAM tensors - out_emb [4096, 412] (gather
layout, token = j*128 + p) and out_sin [4096, 100] (p-major layout,
token = p*32 + j, so each partition stores contiguous multi-KB runs) -
and the host concatenates columns during unsharding.

sin accuracy: x = freq*tok <= 3168 rad. k = RNE(x/2pi) exactly via magic
add/sub, r = ((x - k*c1) - k*c2) - k*c3 (Cody-Waite), clamp to +-PI_SAFE
so the ACT Sin table never sees |x| > pi. Worst case ~2e-4 abs err on the
~6e-5 fraction of elements within float rounding of an odd multiple of pi
(k off by one, |r| marginally > pi, clamped); everywhere else ~4e-7.
"""

import numpy as np

import concourse.bacc as bacc
import concourse.bass as bass
import concourse.mybir as mybir
import concourse.tile as tile
from concourse.bass_utils import run_bass_kernel_spmd

B, S = 16, 2048
V, D, H = 32000, 412, 100
OUT_D = 512
GW = 448                           # gathered row width (256-B aligned >= 412)
N_CORES = 8
P = 128
T_CORE = B * S // N_CORES          # 4096 tokens per core
N_TILES = T_CORE // P              # 32 tiles of 128 tokens
JPP = T_CORE // P                  # sin tokens per partition (p-major), 32

# gather chunks in tiles (sum = 32): small head chunk starts the store lanes
# early, small tail chunks keep the final chain short.
CHUNK_TILES = [2, 4, 4, 4, 4, 4, 4, 3, 2, 1]
# gather stream order: ACT's store chunks (3: tiles 10-14, 5: tiles 18-22)
# delivered early to fill ACT's pre-sin idle window.
CHUNK_ORDER = list(range(len(CHUNK_TILES)))
# sin-pipeline blocks in tiles: descending so the last blocks' range
# reduction, Sin, and sin store are all tiny.
BLK_TILES = [10, 10, 6, 4, 2]
WMAX = 16 * H

# schedule: interleaved emission program. Entries:
#   ("g", chunk_idx)            gather on Pool
#   ("blk", b)                  DVE range-reduction block b
#   ("sin", b)                  ACT Sin for block b
#   ("e", eng, t0, t1)          embed store tiles [t0, t1)
#   ("s", eng, b)               sin store for block b
# Best-found schedule (CoreSim 19600 ns/core vs 32872 baseline): kf of
# block 0 rides the ACT Copy activation (fills ACT's pre-sin idle,
# shortens the DVE chain); sin stores alternate SP/Pool; mid/late embed
# stores are <=2-tile pieces so no lane grabs a multi-us job right before
# a tail-critical Sin or sin store becomes ready; the last three tiles'
# stores spread across SP/Pool/Pool so the three lane tails overlap.
SCHEDULE = (
    [("blk", 0, True), ("sin", 0), ("blk", 1), ("sin", 1), ("blk", 2),
     ("sin", 2), ("blk", 3), ("sin", 3), ("blk", 4), ("sin", 4),
     ("s", "sp", 0), ("s", "pool", 1), ("s", "sp", 2), ("s", "pool", 3),
     ("s", "sp", 4),
     ("e", "sp", 0, 2), ("e", "sp", 2, 6), ("e", "sp", 6, 10),
     ("e", "act", 10, 12), ("e", "act", 12, 14), ("e", "sp", 14, 16),
     ("e", "sp", 16, 18), ("e", "act", 18, 20), ("e", "act", 20, 22),
     ("e", "sp", 22, 24), ("e", "sp", 24, 26), ("e", "act", 26, 28),
     ("e", "pool", 28, 29), ("e", "sp", 29, 30), ("e", "pool", 30, 31),
     ("e", "pool", 31, 32)]
)

_f32 = mybir.dt.float32
_i16 = mybir.dt.int16

_TWO_PI = 2.0 * np.pi
def _split_high(v):
    f = np.float32(v)
    return (f.view(np.uint32) & np.uint32(0xFFFFF000)).view(np.float32)
C1 = float(_split_high(_TWO_PI))
C2 = float(_split_high(_TWO_PI - C1))
C3 = float(np.float32(_TWO_PI - C1 - C2))
INV2PI = float(np.float32(1.0 / _TWO_PI))
MAGIC = float(np.float32(1.5 * 2.0**23))  # RNE quantizer for |y| < 2^22
PI_SAFE = float(np.nextafter(np.float32(np.pi), np.float32(0)))

GATHER_MODE = "v5"
_NC = {}


def _build_nc(mode=None, chunk_tiles=None, blk_tiles=None, schedule=None,
              consts_on_pool=False, chunk_order=None):
    chunk_tiles = chunk_tiles or CHUNK_TILES
    blk_tiles = blk_tiles or BLK_TILES
    schedule = schedule or SCHEDULE
    chunk_order = chunk_order or CHUNK_ORDER
    blk_start = np.cumsum([0] + list(blk_tiles))
    nc = bacc.Bacc("TRN2", target_bir_lowering=False, num_swdge_queues=1)
    # consts: [0:H] freqs, [H:H+JPP] p-major token ids as f32
    consts_t = nc.dram_tensor("consts", [P, H + JPP], _f32,
                              kind="ExternalInput")
    idx_t = nc.dram_tensor("idx16", [P, T_CORE // 16], _i16,
                           kind="ExternalInput")
    table_t = nc.dram_tensor("table", [V, GW], _f32, kind="ExternalInput")
    oemb_t = nc.dram_tensor("out_emb", [T_CORE, D], _f32,
                            kind="ExternalOutput")
    osin_t = nc.dram_tensor("out_sin", [T_CORE, H], _f32,
                            kind="ExternalOutput")

    chunk_start = np.cumsum([0] + list(chunk_tiles))

    with tile.TileContext(nc) as tc:
        with (
            tc.tile_pool(name="const", bufs=1) as cpool,
            tc.tile_pool(name="arena", bufs=1) as apool,
            tc.tile_pool(name="work", bufs=2) as wpool,
        ):
            idx_sb = cpool.tile([P, T_CORE // 16], _i16)
            consts_sb = cpool.tile([P, H + JPP], _f32)
            if consts_on_pool:
                # consts first on Pool: the sin pipeline's gate loads before
                # idx; same-engine SWDGE ordering lets the gathers follow
                # the idx write without a semaphore round-trip.
                nc.gpsimd.dma_start(out=consts_sb[:], in_=consts_t[:])
                nc.gpsimd.dma_start(out=idx_sb[:], in_=idx_t[:])
            else:
                # idx via Pool SWDGE: tiny engine hold, and the gather
                # stream engine owns its own critical input.
                nc.gpsimd.dma_start(out=idx_sb[:], in_=idx_t[:])
                nc.sync.dma_start(out=consts_sb[:], in_=consts_t[:])
            freqs_sb = consts_sb[:, 0:H]
            tokf = consts_sb[:, H : H + JPP]

            ch = apool.tile([P, N_TILES, GW], _f32)    # gathered rows
            r3 = apool.tile([P, JPP, H], _f32)         # clamped angles
            sinout = apool.tile([P, JPP, H], _f32)     # sin values (p-major)
            scratch = apool.tile([P, 1], _f32)

            # ACT warmup: force the Sin act-table load during the idle head.
            nc.vector.memset(scratch[:], 0.0)
            nc.scalar.activation(out=scratch[:], in_=scratch[:],
                                 func=mybir.ActivationFunctionType.Sin)

            def emit_gather(c):
                t0, t1 = chunk_start[c], chunk_start[c + 1]
                toks = (t1 - t0) * P
                nc.gpsimd.dma_gather(
                    ch[:, t0:t1, :],
                    table_t[:],
                    idx_sb[:, t0 * (P // 16) : t1 * (P // 16)],
                    toks, toks, GW,
                )

            def emit_block(b, kf_on_act=False):
                """DVE: x, y, kf, Cody-Waite, clamp for block b.
                kf_on_act routes the kf subtraction through the ACT Copy
                activation (fills ACT's pre-sin idle, shortens DVE)."""
                j0, j1 = blk_start[b], blk_start[b + 1]
                nt = j1 - j0
                w = nt * H
                tb = tokf[:, j0:j1]
                x = wpool.tile([P, WMAX], _f32, tag="x")
                nc.vector.tensor_tensor(
                    out=x[:, 0:w].rearrange("p (j h) -> p j h", j=nt),
                    in0=tb.to_broadcast([P, nt, H]),
                    in1=freqs_sb.rearrange("p (j h) -> p j h", j=1)
                    .to_broadcast([P, nt, H]),
                    op=mybir.AluOpType.mult,
                )
                y = wpool.tile([P, WMAX], _f32, tag="y")
                nc.vector.tensor_scalar(
                    out=y[:, 0:w], in0=x[:, 0:w], scalar1=INV2PI,
                    scalar2=MAGIC,
                    op0=mybir.AluOpType.mult, op1=mybir.AluOpType.add,
                )
                kf = wpool.tile([P, WMAX], _f32, tag="kf")
                if kf_on_act:
                    nc.scalar.activation(
                        out=kf[:, 0:w], in_=y[:, 0:w],
                        func=mybir.ActivationFunctionType.Copy,
                        scale=1.0, bias=-MAGIC,
                    )
                else:
                    nc.vector.tensor_scalar(
                        out=kf[:, 0:w], in0=y[:, 0:w], scalar1=-MAGIC,
                        scalar2=None, op0=mybir.AluOpType.add,
                    )
                r = wpool.tile([P, WMAX], _f32, tag="r")
                nc.vector.cody_waite_cascade(
                    out=r[:, 0:w], x=x[:, 0:w], k=kf[:, 0:w],
                    c1=C1, c2=C2, c3=C3,
                )
                nc.vector.tensor_scalar(
                    out=r3[:, j0:j1, :],
                    in0=r[:, 0:w].rearrange("p (j h) -> p j h", j=nt),
                    scalar1=PI_SAFE, scalar2=-PI_SAFE,
                    op0=mybir.AluOpType.min, op1=mybir.AluOpType.max,
                )

            def emit_sin(b):
                j0, j1 = blk_start[b], blk_start[b + 1]
                nc.scalar.activation(
                    out=sinout[:, j0:j1, :],
                    in_=r3[:, j0:j1, :],
                    func=mybir.ActivationFunctionType.Sin,
                )

            ENG = {"sp": nc.sync, "act": nc.scalar, "pool": nc.gpsimd}

            def emit_sstore(eng, b):
                # p-major: one contiguous multi-KB run per partition
                j0, j1 = blk_start[b], blk_start[b + 1]
                ENG[eng].dma_start(
                    out=osin_t[:].rearrange("(p j) c -> p j c", p=P)
                    [:, j0:j1, :],
                    in_=sinout[:, j0:j1, :],
                )

            def emit_estore(eng, t0, t1):
                ENG[eng].dma_start(
                    out=oemb_t[t0 * P : t1 * P, :]
                    .rearrange("(j p) c -> p j c", p=P),
                    in_=ch[:, t0:t1, 0:D],
                )

            # ---- emission (priority = emission order for the tile
            # scheduler; lane = issuing engine) ----
            for c in chunk_order:
                emit_gather(c)                   # Pool stream
            for step in schedule:
                if step[0] == "blk":
                    emit_block(step[1], *step[2:])
                elif step[0] == "sin":
                    emit_sin(step[1])
                elif step[0] == "e":
                    emit_estore(step[1], step[2], step[3])
                elif step[0] == "s":
                    emit_sstore(step[1], step[2])
                else:
                    raise ValueError(step)
    nc.compile()
    return nc


def _get_nc(mode=None):
    if "v5" not in _NC:
        _NC["v5"] = _build_nc()
    return _NC["v5"]


def make_in_maps(word_ids, word_emb_table, mode=None):
    ids = np.ascontiguousarray(np.asarray(word_ids)).astype(np.int32).reshape(-1)
    table = np.asarray(word_emb_table, dtype=np.float32)
    padded = np.zeros((V, GW), np.float32)
    padded[:, 0:D] = table
    freqs_row = np.arange(H, dtype=np.float32) / np.float32(1000.0)

    in_maps = []
    for c in range(N_CORES):
        shard = ids[c * T_CORE : (c + 1) * T_CORE]
        consts = np.empty((P, H + JPP), np.float32)
        consts[:, 0:H] = freqs_row
        # p-major token layout for the sin pipeline: tok (p, j) = shard[p*JPP+j]
        consts[:, H:] = shard.reshape(P, JPP).astype(np.float32)
        # wrapped int16 layout for dma_gather: token i at [i % 16, i // 16],
        # replicated over the 8 groups of 16 partitions (one per Q7 core)
        base = shard.astype(np.int16).reshape(T_CORE // 16, 16).T  # [16, n/16]
        in_maps.append({
            "consts": consts,
            "table": padded,
            "idx16": np.ascontiguousarray(np.tile(base, (8, 1))),
        })
    return in_maps


def kernel(word_ids, word_emb_table):
    nc = _get_nc()
    in_maps = make_in_maps(word_ids, word_emb_table)
    res = run_bass_kernel_spmd(nc, in_maps, core_ids=list(range(N_CORES)))
    outs = []
    for r in res.results:
        emb = r["out_emb"]                       # [T_CORE, 412], token-major
        sin = r["out_sin"]                       # [T_CORE, 100], token-major
        outs.append(np.concatenate([emb, sin], axis=1))
    return np.concatenate(outs, axis=0).reshape(B, S, OUT_D)
